# revision 50
# baseline (speedup 1.0000x reference)
"""LyraGemma3 sliding-window attention — Trainium2 Bass kernel, 8 NeuronCores.

Sharding: core = b*4 + h (b batch, h head-group). Each core owns vanilla head
h, lyra head 4+h, kv head h for batch b and produces output rows
[512h, 512h+512) of batch b. No collectives.

v3 vs v2 (~353us -> ~312us un-throttled):
- softmax denominators accumulated on the DVE into SBUF (acc += probs per
  key tile, emitted with the lagged po stage so the in-order vector stream
  never waits on scalar) and reduced with ONE 512-row colsum matmul per
  (Q,stream) instead of a per-tile sum matmul (-12us PE);
- scores/mask/exp narrowed to the true valid column range; the T_lo po
  seed stays full width (psum start=True zeroes the whole 2KB bank row) but
  reads gpsimd-memset zero probs, so the wasted scores/exp work is gone;
- mask add narrowed to the single partial 128-col boundary block; two
  shared [128,128] triangle masks replace the 8x[128,512] mask bank;
- po matmuls lag 3 key-tiles behind scores (psS 3 bufs, probs 4 bufs) to
  hide the mask+exp chain from the in-order PE;
- per-(Q,stream) finalize split into two closures (colsum+rstd, bcast+mul)
  interleaved into the next stream/phase's emission so the PE never idles
  on the scalar Ln/Exp round-trip;
- k-stats via 1-column matmuls (transposed per-token ssq, -4us PE), k
  psum->sbuf copies land directly in kTn (gpsimd copies removed);
- all DRAM inputs host-packed fully contiguous (>=2KB/partition lines);
  hst on the sync HWDGE ring, weights on the scalar ring, wo prefetch
  deferred past the startup-critical window; persistent operands packed
  into few wide tiles to shrink the semaphore count;
- phase D: kc-paired [128,1024] wo tiles, psum double-buffered, m-outer
  last block so the final flush is one copy+DMA, output DMAs on the
  scalar ring;
- pre-D fill: phase D's first two m-groups of co block 0 (dependent only
  on C0/C1-era outC and resident cb0 wo tiles) run inside the attention
  pool scope on dead pss-ring psum banks, interleaved with the last
  pended finalize — the PE never idles across the C->D transition.
"""

import sys

sys.path.insert(0, "/opt/trn_rl_repo")

import numpy as np
import ml_dtypes

import concourse.bass as bass
import concourse.tile as tile
from concourse import mybir
from concourse.tile import ScopedClock

F32 = mybir.dt.float32
F32R = mybir.dt.float32r
BF16 = mybir.dt.bfloat16
AF = mybir.ActivationFunctionType

B, S, HID = 2, 2048, 2560
H, KV, D = 8, 4, 256
WINDOW = 1024
THETA = 10000.0
EPS = 1e-6
SCALING = 256.0 ** (-0.5)  # 1/16

NKC = HID // 128  # 20 contraction chunks for projections
NST = 8           # s-tiles of 256 tokens
NT = S // 128     # 16 key tiles of 128
NQ = 4            # attention q-tiles of 512
MASK_NEG = -1e30


class SplitWaitTC(tile.TileContext):
    """This container's walrus encodes at most ONE semaphore wait per
    instruction; Tile emits multi-wait sync_info. Hoist extra waits onto
    preceding same-engine NOPs."""

    def _drain_and_barrier(self, tick_clock, wait_clock):
        nc = self.nc
        drain_inst = nc.sync.drain()
        wait_clock.add_sem_waits(
            drain_inst.ins, ScopedClock({None: tick_clock.global_clock})
        )
        self._split_multi_waits()
        nc.all_engine_barrier()
        popped = nc._tile_sem_poison_stack.pop()
        assert popped is self._sem_poison
        nc.clear_and_free_semaphores(list(self.sems.allocated().values()))
        nc.all_engine_barrier()

    def _split_multi_waits(self):
        nc = self.nc
        cur_bb = nc.cur_bb
        assert cur_bb is not None
        for f in nc.m.functions:
            for blk in f.blocks:
                insts = blk.instructions
                i = 0
                while i < len(insts):
                    inst = insts[i]
                    si = inst.sync_info
                    if si is not None and si.on_wait and len(si.on_wait) > 1:
                        waits = list(si.on_wait)
                        inst.sync_info = mybir.SyncInfo(
                            on_wait=waits[-1:], on_update=si.on_update
                        )
                        eng = inst.engine
                        for w in waits[:-1]:
                            nop = nc.engines[eng].nop()
                            nop.ins.sync_info = mybir.SyncInfo(
                                on_wait=[w], on_update=[]
                            )
                            cur_bb.bb.instructions.remove(nop.ins)
                            insts.insert(i, nop.ins)
                            i += 1
                    i += 1


def _mask_index(T, Q):
    """Mask tile for key-tile T against q-tile Q (queries [512Q,512Q+512)).
    Returns None (fully valid), 4+j (causal), or j'' (window edge)."""
    j = T - 4 * Q
    if j >= 0:
        return 4 + j
    if T >= 4 * Q - 4:
        return None
    return T - (4 * Q - 8)


def _col_range(midx):
    """True valid query-column range for a key tile."""
    if midx is None:
        return 0, 512
    if midx >= 4:
        return 128 * (midx - 4), 512
    return 0, 128 * (midx + 1)


def build_program():
    nc = bass.Bass()

    hsP = nc.declare_dram_parameter("hsP", [NST * 128, NKC * 256], BF16, isOutput=False)
    wqP = nc.declare_dram_parameter("wqP", [128, NKC * 512], BF16, isOutput=False)
    wkP = nc.declare_dram_parameter("wkP", [128, NKC * 256], BF16, isOutput=False)
    wvP = nc.declare_dram_parameter("wvP", [128, NKC * 256], BF16, isOutput=False)
    woP = nc.declare_dram_parameter("woP", [128, 40 * 1024], BF16, isOutput=False)
    cos_d = nc.declare_dram_parameter("cos_t", [128, S], BF16, isOutput=False)
    sin_d = nc.declare_dram_parameter("sin_t", [128, S], BF16, isOutput=False)
    masks_d = nc.declare_dram_parameter("masks2", [128, 256], BF16, isOutput=False)
    invq_d = nc.declare_dram_parameter("invq", [128, 2], BF16, isOutput=False)
    invk_d = nc.declare_dram_parameter("invk", [128, 2], BF16, isOutput=False)
    onec_d = nc.declare_dram_parameter("onec", [128, 1], BF16, isOutput=False)
    oner_d = nc.declare_dram_parameter("oner", [1, 128], F32R, isOutput=False)
    epsb_d = nc.declare_dram_parameter("epsb", [128, 2], F32, isOutput=False)
    out_d = nc.declare_dram_parameter("out", [512, HID], F32, isOutput=True)

    with SplitWaitTC(nc) as tc:
        with (
            tc.tile_pool(name="outer", bufs=1) as pO,
            tc.tile_pool(name="wo_pool", bufs=1) as pWo,
            tc.tile_pool(name="hst_pool", bufs=2) as pH,
        ):
            # ---- persistent SBUF tensors -------------------------------
            hst_t = {}

            def load_hst(st, pieces=(NKC,)):
                t = pH.tile([128, NKC * 256], BF16, name="hst")
                hst_t[st] = t
                c0 = 0
                for step in pieces:
                    nc.sync.dma_start(
                        t[:, c0 * 256 : (c0 + step) * 256],
                        hsP[st * 128 : (st + 1) * 128, c0 * 256 : (c0 + step) * 256],
                    )
                    c0 += step

            # startup-critical loads: hst(0) in pieces on the sync ring,
            # weights stream in parallel on the scalar ring
            load_hst(0, pieces=(2, 3, 5, 5, 5))
            wq_sb = pO.tile([128, NKC * 512], BF16, name="wq_sb")
            wq_pieces = [(0, 1), (1, 2), (3, 3), (6, 7), (13, 7)]
            for c0, w in wq_pieces:
                nc.scalar.dma_start(
                    wq_sb[:, c0 * 512 : (c0 + w) * 512],
                    wqP[:, c0 * 512 : (c0 + w) * 512],
                )
            wk_sb = pO.tile([128, NKC * 256], BF16, name="wk_sb")
            for pc in range(2):
                nc.scalar.dma_start(
                    wk_sb[:, pc * 10 * 256 : (pc + 1) * 10 * 256],
                    wkP[:, pc * 10 * 256 : (pc + 1) * 10 * 256],
                )
            wv_sb = pO.tile([128, NKC * 256], BF16, name="wv_sb")
            for pc in range(2):
                nc.scalar.dma_start(
                    wv_sb[:, pc * 10 * 256 : (pc + 1) * 10 * 256],
                    wvP[:, pc * 10 * 256 : (pc + 1) * 10 * 256],
                )
            # cos/sin next: phase A's first rope needs them at ~18us
            cos_sb = pO.tile([128, S], BF16, name="cos_sb")
            nc.sync.dma_start(cos_sb[:], cos_d[:])
            sin_sb = pO.tile([128, S], BF16, name="sin_sb")
            nc.sync.dma_start(sin_sb[:], sin_d[:])
            invq = pO.tile([128, 2], BF16, name="invq")
            nc.sync.dma_start(invq[:], invq_d[:])
            invk = pO.tile([128, 2], BF16, name="invk")
            nc.sync.dma_start(invk[:], invk_d[:])
            onec = pO.tile([128, 1], BF16, name="onec")
            nc.sync.dma_start(onec[:], onec_d[:])
            oner = pO.tile([1, 128], F32R, name="oner")
            nc.sync.dma_start(oner[:], oner_d[:])
            epsb = pO.tile([128, 2], F32, name="epsb")
            nc.sync.dma_start(epsb[:], epsb_d[:])
            load_hst(1)
            masks_sb = pO.tile([128, 256], BF16, name="masks_sb")
            nc.sync.dma_start(masks_sb[:], masks_d[:])
            maskC = masks_sb[:, 0:128]    # causal boundary: valid iff y >= x
            maskW = masks_sb[:, 128:256]  # window boundary: valid iff y < x

            # persistent attention operands (written by A, read by C);
            # packed into few wide tiles to keep the semaphore count (and
            # with it the end-of-program clear sequence) small
            qTall = pO.tile([128, 4 * S], BF16, name="qTall")
            qT = [qTall[:, c * S : (c + 1) * S] for c in range(4)]
            kTall = pO.tile([128, 4 * S], BF16, name="kTall")
            kTr = [kTall[:, c * S : (c + 1) * S] for c in range(2)]
            kTn = [kTall[:, (2 + c) * S : (3 + c) * S] for c in range(2)]
            vA = pO.tile([128, NT * 256], BF16, name="vA")
            krstdA = pO.tile([128, 2 * NST], F32, name="krstdA")
            outCall = pO.tile([128, 4 * S], BF16, name="outCall")
            outC = [
                [outCall[:, (2 * s + c) * S : (2 * s + c + 1) * S] for c in range(2)]
                for s in range(2)
            ]

            # wo prefetch tiles: kc-paired [128,1024] => 2KB/partition lines.
            # DMAs are emitted inside the phase loop (5 per A phase) so their
            # triggers/bandwidth don't crowd out the startup-critical loads.
            wosb_t = {}
            co_blocks = [(cb * 512, 512) for cb in range(5)]

            def load_wosb(st):
                # defer wo prefetch past the startup-critical window: no
                # triggers/bandwidth before phase A(2)
                if st < 2 or st > 6:
                    return
                for idx in range((st - 2) * 8, (st - 2) * 8 + 8):
                    cb, kcp = idx // 8, idx % 8
                    t = pWo.tile([128, 1024], BF16, name="wosb", bufs=10)
                    nc.sync.dma_start(
                        t[:],
                        woP[:, (cb * 8 + kcp) * 1024 : (cb * 8 + kcp + 1) * 1024],
                    )
                    wosb_t[(cb, kcp)] = t

            with (
                tc.tile_pool(name="pA", bufs=2) as pA,
                tc.tile_pool(name="pAs", bufs=2) as pAs,
                tc.tile_pool(name="pC", bufs=4) as pC,
                tc.tile_pool(name="psA", bufs=2, space="PSUM") as psA,
                tc.tile_pool(name="psSm", bufs=1, space="PSUM") as psSm,
                tc.tile_pool(name="psS", bufs=3, space="PSUM") as psS,
                tc.tile_pool(name="psPo", bufs=1, space="PSUM") as psPo,
            ):

                def sm_tile():
                    return psSm.tile([128, 512], F32, name="sm")

                pending_fin = []

                def flush_one():
                    if pending_fin:
                        pending_fin.pop(0)()

                def flush_fin():
                    while pending_fin:
                        pending_fin.pop(0)()

                def phase_a(st):
                    s0 = st * 256
                    if st + 2 < NST:
                        load_hst(st + 2)
                    load_wosb(st)
                    hst = hst_t[st]
                    # ---- q projection: 2 psum tiles of 2 d-chunks each.
                    # kc-outer so wq is consumed in DMA-arrival order at the
                    # arrival rate (matters for the A0 startup ramp)
                    pq2 = [psA.tile([128, 512], F32, name="pacc") for _ in range(2)]
                    for kc in range(NKC):
                        for half in range(2):
                            for g in range(2):
                                chunk = half * 2 + g
                                nc.tensor.matmul(
                                    pq2[half][:, g * 256 : (g + 1) * 256],
                                    wq_sb[
                                        :,
                                        kc * 512 + chunk * 128 : kc * 512
                                        + (chunk + 1) * 128,
                                    ],
                                    hst[:, kc * 256 : (kc + 1) * 256],
                                    # start only once per bank: its pending
                                    # mark covers g=1's first write too
                                    start=(kc == 0 and g == 0),
                                    stop=(kc == NKC - 1),
                                    skip_group_check=True,
                                )
                    qz = []
                    for half in range(2):
                        z = pA.tile([128, 512], BF16, name=f"qz{half}")
                        nc.scalar.activation(z[:], pq2[half][:], AF.Copy)
                        qz.append(z)
                    flush_one()
                    sqt = pAs.tile([128, 1024], BF16, name="sqt")
                    sq0, sq1 = sqt[:, 0:512], sqt[:, 512:1024]
                    nc.gpsimd.tensor_mul(sq0, qz[0][:], qz[0][:])
                    nc.gpsimd.tensor_mul(sq1, qz[1][:], qz[1][:])
                    # ---- k projection (psum->sbuf copy lands directly in kTn)
                    pk = psA.tile([128, 512], F32, name="pacc")
                    for g in range(2):
                        for kc in range(NKC):
                            nc.tensor.matmul(
                                pk[:, g * 256 : (g + 1) * 256],
                                wk_sb[
                                    :, kc * 256 + g * 128 : kc * 256 + (g + 1) * 128
                                ],
                                hst[:, kc * 256 : (kc + 1) * 256],
                                start=(kc == 0),
                                stop=(kc == NKC - 1),
                                skip_group_check=True,
                            )
                    nc.scalar.activation(
                        kTn[0][:, s0 : s0 + 256], pk[:, 0:256], AF.Copy
                    )
                    nc.scalar.activation(
                        kTn[1][:, s0 : s0 + 256], pk[:, 256:512], AF.Copy
                    )
                    flush_one()
                    sqk = pAs.tile([128, 512], BF16, name="sqk")
                    nc.gpsimd.tensor_mul(
                        sqk[:, 0:256],
                        kTn[0][:, s0 : s0 + 256],
                        kTn[0][:, s0 : s0 + 256],
                    )
                    nc.gpsimd.tensor_mul(
                        sqk[:, 256:512],
                        kTn[1][:, s0 : s0 + 256],
                        kTn[1][:, s0 : s0 + 256],
                    )
                    # q-stats: Ln/Exp round-trip overlaps the v projection
                    pnq = sm_tile()
                    for h in range(2):
                        sq = sq0 if h == 0 else sq1
                        for c in range(2):
                            nc.tensor.matmul(
                                pnq[0:1, h * 256 : (h + 1) * 256],
                                invq[:, c : c + 1],
                                sq[:, c * 256 : (c + 1) * 256],
                                start=(c == 0),
                                stop=(c == 1),
                                skip_group_check=True,
                            )
                    lnq = pAs.tile([1, 512], F32, name="lnq")
                    nc.scalar.activation(
                        lnq[:],
                        pnq[0:1, :],
                        AF.Ln,
                        bias=epsb[0:1, 0:1],
                        scale=1.0 / 256.0,
                    )
                    rstdq = pAs.tile([1, 512], F32R, name="rstdq")
                    nc.scalar.activation(rstdq[:], lnq[:], AF.Exp, scale=-0.5)
                    # ---- v projection (tokens on partitions) ----
                    pv = psA.tile([128, 512], F32, name="pacc")
                    for sm in range(2):
                        for kc in range(NKC):
                            nc.tensor.matmul(
                                pv[:, sm * 256 : (sm + 1) * 256],
                                hst[:, kc * 256 + sm * 128 : kc * 256 + sm * 128 + 128],
                                wv_sb[:, kc * 256 : (kc + 1) * 256],
                                start=(kc == 0),
                                stop=(kc == NKC - 1),
                                skip_group_check=True,
                            )
                    nc.scalar.activation(
                        vA[:, st * 512 : (st + 1) * 512], pv[:], AF.Copy
                    )
                    # k-stats: per-token ssq via 1-column matmuls (transposed)
                    pnk = sm_tile()
                    for th in range(2):  # token half of this s-tile
                        for c in range(2):  # d-chunk
                            nc.tensor.matmul(
                                pnk[:, th : th + 1],
                                sqk[:, c * 256 + th * 128 : c * 256 + th * 128 + 128],
                                invk[:, c : c + 1],
                                start=(c == 0),
                                stop=(c == 1),
                                skip_group_check=True,
                            )
                    lnk = pAs.tile([128, 2], F32, name="lnk")
                    nc.scalar.activation(
                        lnk[:], pnk[:, 0:2], AF.Ln, bias=epsb[:, 1:2]
                    )
                    nc.scalar.activation(
                        krstdA[:, 2 * st : 2 * st + 2], lnk[:], AF.Exp, scale=-0.5
                    )
                    pbcq = sm_tile()
                    nc.tensor.matmul(pbcq[:], oner[:], rstdq[:], start=True, stop=True)
                    bcs = pAs.tile([128, 512], BF16, name="bcs")
                    nc.scalar.activation(bcs[:], pbcq[:], AF.Copy)

                    # ---- rope (bf16 on DVE) ----
                    cs = cos_sb[:, s0 : s0 + 256]
                    sn = sin_sb[:, s0 : s0 + 256]

                    def rope2(z0, z1, bc, d0, d1):
                        rt = pA.tile([128, 6 * 256], BF16, name="ropet")
                        t0, t1, u0 = rt[:, 0:256], rt[:, 256:512], rt[:, 512:768]
                        t2, t3, u1 = rt[:, 768:1024], rt[:, 1024:1280], rt[:, 1280:1536]
                        nc.vector.tensor_mul(t0, z0, cs)
                        nc.vector.tensor_mul(t1, z1, sn)
                        nc.vector.tensor_sub(u0, t0, t1)
                        nc.vector.tensor_mul(t2, z1, cs)
                        nc.vector.tensor_mul(t3, z0, sn)
                        if bc is not None:
                            nc.vector.tensor_add(u1, t2, t3)
                            nc.vector.tensor_mul(d0, u0, bc)
                            nc.vector.tensor_mul(d1, u1, bc)
                        else:
                            nc.vector.tensor_copy(d0, u0)
                            nc.vector.tensor_add(d1, t2, t3)

                    for h in range(2):
                        rope2(
                            qz[h][:, 0:256],
                            qz[h][:, 256:512],
                            bcs[:, h * 256 : (h + 1) * 256],
                            qT[2 * h][:, s0 : s0 + 256],
                            qT[2 * h + 1][:, s0 : s0 + 256],
                        )
                    rope2(
                        kTn[0][:, s0 : s0 + 256],
                        kTn[1][:, s0 : s0 + 256],
                        None,
                        kTr[0][:, s0 : s0 + 256],
                        kTr[1][:, s0 : s0 + 256],
                    )

                def phase_c(Q, po_from_pacc=None):
                    fin0 = None
                    accQ = pC.tile([128, 1024], F32, name="accQ", bufs=2)
                    accbQ = pC.tile([128, 1024], BF16, name="accbQ", bufs=2)
                    for stream in range(2):
                        kT = kTr if stream == 0 else kTn
                        q0 = qT[2 * stream]
                        q1 = qT[2 * stream + 1]
                        T_lo = max(0, 4 * Q - 8)
                        T_hi = 4 * Q + 3
                        if po_from_pacc is not None and stream == po_from_pacc:
                            po0 = psA.tile([128, 512], F32, name="pacc")
                            po1 = psA.tile([128, 512], F32, name="pacc")
                        else:
                            po0 = psPo.tile([128, 512], F32, name="po0")
                            po1 = psPo.tile([128, 512], F32, name="po1")
                        acc = accQ[:, stream * 512 : (stream + 1) * 512]
                        # stage stream0's finalize PE work into stream1's
                        # emission (hides the acc->accb->colsum->rstd chain);
                        # a leftover pended closure drains into stream0
                        if stream == 0:
                            interject = {1: [flush_one], 4: [flush_one]}
                        else:
                            interject = {2: [fin0[0]], 5: [fin0[1]]}

                        def emit_scores(T):
                            midx = _mask_index(T, Q)
                            c0, c1 = _col_range(midx)
                            pss = psS.tile([128, 512], F32, name="pss")
                            nc.tensor.matmul(
                                pss[:, c0:c1],
                                kT[0][:, T * 128 : (T + 1) * 128],
                                q0[:, Q * 512 + c0 : Q * 512 + c1],
                                start=True,
                                stop=False,
                            )
                            nc.tensor.matmul(
                                pss[:, c0:c1],
                                kT[1][:, T * 128 : (T + 1) * 128],
                                q1[:, Q * 512 + c0 : Q * 512 + c1],
                                start=False,
                                stop=True,
                            )
                            if midx is not None:
                                if midx >= 4:
                                    mb0, msk = c0, maskC
                                else:
                                    mb0, msk = c1 - 128, maskW
                                nc.vector.tensor_add(
                                    pss[:, mb0 : mb0 + 128],
                                    pss[:, mb0 : mb0 + 128],
                                    msk,
                                )
                            probs = pC.tile([128, 512], BF16, name="probs")
                            nc.scalar.activation(
                                probs[:, c0:c1],
                                pss[:, c0:c1],
                                AF.Exp,
                                scale=krstdA[:, T : T + 1],
                            )
                            if T == T_lo and c1 != 512:
                                # psum start=True zeroes a whole bank row, so
                                # the T_lo po matmuls must seed full width —
                                # zero the invalid probs columns they read
                                nc.gpsimd.memset(probs[:, c1:512], 0.0)
                            return (T, c0, c1, probs)

                        def emit_po(item, last):
                            T, c0, c1, probs = item
                            first = T == T_lo
                            a, bnd = (0, 512) if first else (c0, c1)
                            nc.tensor.matmul(
                                po0[:, a:bnd],
                                vA[:, T * 256 : T * 256 + 128],
                                probs[:, a:bnd],
                                start=first,
                                stop=last,
                                skip_group_check=True,
                            )
                            nc.tensor.matmul(
                                po1[:, a:bnd],
                                vA[:, T * 256 + 128 : T * 256 + 256],
                                probs[:, a:bnd],
                                start=first,
                                stop=last,
                                skip_group_check=True,
                            )
                            if first:
                                nc.vector.tensor_copy(acc[:], probs[:])
                            else:
                                nc.vector.tensor_add(
                                    acc[:, c0:c1], acc[:, c0:c1], probs[:, c0:c1]
                                )

                        LAG = 3
                        pending = []
                        for i, T in enumerate(range(T_lo, T_hi + 1)):
                            pending.append(emit_scores(T))
                            for f in interject.pop(i, ()):
                                f()
                            if len(pending) > LAG:
                                emit_po(pending.pop(0), last=False)
                        while pending:
                            emit_po(pending.pop(0), last=(len(pending) == 0))
                        for i in sorted(interject):
                            for f in interject.pop(i):
                                f()

                        # ---- finalize (two staged closures) ----
                        accb = accbQ[:, stream * 512 : (stream + 1) * 512]
                        nc.scalar.activation(accb, acc, AF.Copy)

                        def fin_colsum(accb=accb):
                            psm = sm_tile()[0:1, :]
                            nc.tensor.matmul(
                                psm[:], onec[:], accb[:], start=True, stop=True
                            )
                            lnm = pC.tile([1, 512], F32, name="lnm", bufs=2)
                            nc.scalar.activation(lnm[:], psm[:], AF.Ln)
                            rstCr = pC.tile([1, 512], F32R, name="rstCr", bufs=2)
                            nc.scalar.activation(rstCr[:], lnm[:], AF.Exp, scale=-1.0)
                            return rstCr

                        def fin_bc(stream=stream, po0=po0, po1=po1, get=None):
                            rstCr = get()
                            pbc = sm_tile()
                            nc.tensor.matmul(
                                pbc[:], oner[:], rstCr[:], start=True, stop=True
                            )
                            bcsC = pC.tile(
                                [128, 512], BF16, name="bcsC", bufs=2
                            )
                            nc.scalar.activation(bcsC[:], pbc[:], AF.Copy)
                            with nc.allow_low_precision(reason="attn out bf16"):
                                nc.vector.tensor_mul(
                                    outC[stream][0][:, Q * 512 : (Q + 1) * 512],
                                    po0[:],
                                    bcsC[:],
                                )
                                nc.vector.tensor_mul(
                                    outC[stream][1][:, Q * 512 : (Q + 1) * 512],
                                    po1[:],
                                    bcsC[:],
                                )

                        box = {}

                        def stage1(box=box, fin_colsum=fin_colsum):
                            box["rstCr"] = fin_colsum()

                        def stage2(box=box, fin_bc=fin_bc):
                            fin_bc(get=lambda: box["rstCr"])

                        if stream == 0:
                            fin0 = (stage1, stage2)
                        else:
                            pending_fin.append(stage1)
                            pending_fin.append(stage2)

                # interleave: A(0),A(1),C(0),A(2),A(3),C(1),A(4)..A(7)
                for st in range(NST):
                    phase_a(st)
                    if st in (1, 3):
                        phase_c(st // 2)
                # post-A attention: second stream borrows pacc banks
                phase_c(2, po_from_pacc=1)
                phase_c(3, po_from_pacc=1)

                # ---- pre-D: fill the final-finalize PE hole with phase D's
                # first two m-groups of co block 0 (they only read C0/C1-era
                # outC tokens and the long-resident cb0 wo tiles), using the
                # dead pss ring banks as psum. C3-stream1's pended finalize
                # stages drain between the matmul groups.
                def d_matmul_pre(pos_m, m, kc, first, last):
                    wosb = wosb_t[(0, kc // 2)]
                    dc = kc % 2
                    stream, m0 = m // 2, (m % 2) * 128
                    lhsT = outC[stream][dc][:].rearrange(
                        "p (mm j) -> p mm j", j=8
                    )[:, m0 : m0 + 128, kc // 2 : kc // 2 + 1]
                    nc.tensor.matmul(
                        pos_m[:],
                        lhsT,
                        wosb[:, dc * 512 : (dc + 1) * 512],
                        start=first,
                        stop=last,
                    )

                posA = psS.tile([128, 512], F32, name="pss")
                for kc in range(12):
                    d_matmul_pre(posA, 0, kc, kc == 0, False)
                flush_one()
                for kc in range(12, 16):
                    d_matmul_pre(posA, 0, kc, False, kc == 15)
                posB = psS.tile([128, 512], F32, name="pss")
                for kc in range(4):
                    d_matmul_pre(posB, 2, kc, kc == 0, False)
                flush_one()
                ostA = pC.tile([128, 512], F32, name="ostPre", bufs=2)
                nc.scalar.activation(ostA[:], posA[:], AF.Copy)
                nc.scalar.dma_start(out_d[0:128, 0:512], ostA[:])
                for kc in range(4, 16):
                    d_matmul_pre(posB, 2, kc, False, kc == 15)
                ostB = pC.tile([128, 512], F32, name="ostPre", bufs=2)
                nc.scalar.activation(ostB[:], posB[:], AF.Copy)
                nc.scalar.dma_start(out_d[256:384, 0:512], ostB[:])
                flush_fin()

            # ================= PHASE D: output projection ================
            with (
                tc.tile_pool(name="pD", bufs=3) as pD,
                tc.tile_pool(name="pDps", bufs=2, space="PSUM") as psD,
            ):
                def d_flush(m, pos_m, co):
                    ost = pD.tile([128, 512], F32, name="ost")
                    if m % 2 == 0:
                        nc.scalar.activation(ost[:], pos_m[:], AF.Copy)
                    else:
                        nc.vector.tensor_copy(ost[:], pos_m[:])
                    nc.scalar.dma_start(
                        out_d[m * 128 : (m + 1) * 128, co : co + 512], ost[:]
                    )

                def d_matmul(pos_m, m, cb, kc, first, last):
                    wosb = wosb_t[(cb, kc // 2)]
                    j, dc = kc // 2, kc % 2
                    stream, m0 = m // 2, (m % 2) * 128
                    lhsT = outC[stream][dc][:].rearrange(
                        "p (mm j) -> p mm j", j=8
                    )[:, m0 : m0 + 128, j : j + 1]
                    nc.tensor.matmul(
                        pos_m[:],
                        lhsT,
                        wosb[:, dc * 512 : (dc + 1) * 512],
                        start=first,
                        stop=last,
                    )

                for cb, (co, w) in enumerate(co_blocks):
                    if cb < 4:
                        # kc-outer: wosb ring-friendly, psD bufs=2 overlaps
                        # the flush with the next block's matmuls. cb0's
                        # m=0/m=2 groups already ran in the pre-D fill.
                        ms = [1, 3] if cb == 0 else [0, 1, 2, 3]
                        pos = {
                            m: psD.tile([128, 512], F32, name=f"pD{m}")
                            for m in ms
                        }
                        for kc in range(16):
                            for m in ms:
                                d_matmul(pos[m], m, cb, kc, kc == 0, kc == 15)
                        for m in ms:
                            d_flush(m, pos[m], co)
                    else:
                        # last block: m-outer so the final flush is 1 copy+DMA
                        for m in range(4):
                            pos_m = psD.tile([128, 512], F32, name=f"pD{m}")
                            for kc in range(16):
                                d_matmul(pos_m, m, cb, kc, kc == 0, kc == 15)
                            if m < 3:
                                d_flush(m, pos_m, co)
                            else:
                                # split the very last flush so its first DMA
                                # overlaps the second half's copy
                                ost = pD.tile([128, 512], F32, name="ost")
                                nc.scalar.activation(
                                    ost[:, 0:256], pos_m[:, 0:256], AF.Copy
                                )
                                nc.scalar.dma_start(
                                    out_d[384:512, co : co + 256], ost[:, 0:256]
                                )
                                nc.vector.tensor_copy(
                                    ost[:, 256:512], pos_m[:, 256:512]
                                )
                                nc.scalar.dma_start(
                                    out_d[384:512, co + 256 : co + 512],
                                    ost[:, 256:512],
                                )
    return nc


def _host_inputs(hidden_states, wq, wk, wv, wo, q_norm_w, k_norm_w):
    """Build the 8 per-core input maps (all host-side numpy prep)."""
    bf16 = ml_dtypes.bfloat16
    hs = np.asarray(hidden_states, dtype=np.float32)
    wq = np.asarray(wq, dtype=np.float32)
    wk = np.asarray(wk, dtype=np.float32)
    wv = np.asarray(wv, dtype=np.float32)
    wo = np.asarray(wo, dtype=np.float32)
    qnw = np.asarray(q_norm_w, dtype=np.float32)
    knw = np.asarray(k_norm_w, dtype=np.float32)

    # hsP[st*128+p, kc*256+s] = hs[b].T[kc*128+p, st*256+s]  (contiguous tiles)
    hsP = [
        np.ascontiguousarray(
            hs[b].T.astype(bf16)
            .reshape(NKC, 128, NST, 256)
            .transpose(2, 1, 0, 3)
            .reshape(NST * 128, NKC * 256)
        )
        for b in range(B)
    ]

    # woP[p, (cb*8+kcp)*1024 + b*512 + j] = wo[(2*kcp+b)*128+p, cb*512+j]
    woP = np.ascontiguousarray(
        wo.astype(bf16)
        .reshape(8, 2, 128, 5, 512)
        .transpose(2, 3, 0, 1, 4)
        .reshape(128, 40 * 1024)
    )

    inv_freq = 1.0 / (THETA ** (np.arange(0, D, 2, dtype=np.float32) / D))
    ang = np.outer(inv_freq, np.arange(S, dtype=np.float32))  # (128, S)
    cos_t = np.ascontiguousarray(np.cos(ang)).astype(bf16)
    sin_t = np.ascontiguousarray(np.sin(ang)).astype(bf16)

    # two shared boundary-block masks [128,128]:
    #   causal (diag):  valid iff y >= x ; window edge: valid iff y < x
    x = np.arange(128)[:, None]
    y = np.arange(128)[None, :]
    masks2 = np.empty((128, 256), np.float32)
    masks2[:, 0:128] = np.where(y >= x, 0.0, MASK_NEG)
    masks2[:, 128:256] = np.where(y < x, 0.0, MASK_NEG)
    masks2 = np.ascontiguousarray(masks2).astype(bf16)

    invq = np.ascontiguousarray(
        ((1.0 + qnw) ** -2).reshape(2, 128).T.astype(bf16)
    )
    invk = np.ascontiguousarray(
        ((1.0 + knw) ** -2).reshape(2, 128).T.astype(bf16)
    )
    onec = np.ones((128, 1), bf16)
    oner = np.ones((1, 128), np.float32)

    qs = 1.0 + qnw
    ks = 1.0 + knw
    in_maps = []
    for core in range(8):
        b, h = core // 4, core % 4
        wq2 = np.concatenate(
            [
                wq[:, h * D : (h + 1) * D] * qs[None, :],
                wq[:, (4 + h) * D : (5 + h) * D] * qs[None, :],
            ],
            axis=1,
        )
        wqP = np.ascontiguousarray(
            wq2.astype(bf16).reshape(NKC, 128, 512).transpose(1, 0, 2)
            .reshape(128, NKC * 512)
        )
        wk1 = (wk[:, h * D : (h + 1) * D] * ks[None, :]).astype(bf16)
        wkP = np.ascontiguousarray(
            wk1.reshape(NKC, 128, 256).transpose(1, 0, 2).reshape(128, NKC * 256)
        )
        wv1 = wv[:, h * D : (h + 1) * D].astype(bf16)
        wvP = np.ascontiguousarray(
            wv1.reshape(NKC, 128, 256).transpose(1, 0, 2).reshape(128, NKC * 256)
        )
        in_maps.append(
            {
                "hsP": hsP[b],
                "wqP": wqP,
                "wkP": wkP,
                "wvP": wvP,
                "woP": woP,
                "cos_t": cos_t,
                "sin_t": sin_t,
                "masks2": masks2,
                "invq": invq,
                "invk": invk,
                "onec": onec,
                "oner": oner,
                "epsb": np.tile(np.array([[EPS, 256.0 * EPS]], np.float32), (128, 1)),
            }
        )
    return in_maps


_PROGRAM = None


def kernel(hidden_states, wq, wk, wv, wo, q_norm_w, k_norm_w):
    global _PROGRAM
    from concourse.bass_utils import run_bass_kernel_spmd

    if _PROGRAM is None:
        _PROGRAM = build_program()
    in_maps = _host_inputs(hidden_states, wq, wk, wv, wo, q_norm_w, k_norm_w)
    res = run_bass_kernel_spmd(_PROGRAM, in_maps, core_ids=list(range(8)))
    out = np.empty((B, S, HID), np.float32)
    for core in range(8):
        b, h = core // 4, core % 4
        out[b, h * 512 : (h + 1) * 512, :] = res.results[core]["out"]
    return out
